# revision 4
# baseline (speedup 1.0000x reference)
"""DriftAwareMultiHeadAttention on 8 Trainium2 NeuronCores.

Sharding (per spec hint): core c -> (batch b = c//2, head-group hg = c%2).
Each core: fp16 QKV projection (column-parallel over its 8 heads), full
attention for those heads, row-parallel partial output projection.
Host gather: y[b] = (yT[2b] + yT[2b+1]).T + b_out.

Measured on HW: ~400-405 us/core (vs 415-436 us baseline), rel err 9.4e-3.

Layout: feature-on-partition / token-on-free throughout (no on-chip
transposes).  Q^T/K^T [512, 2048] fp16 (head h -> e-tile h//2, partition
offset (h%2)*64); V [tokens, 8 heads x (64+1)] fp16 with a ones column so
the AV matmul emits the softmax denominator for free.

Phase 1 -- projections: fp16 chains (8 contraction chunks, N=512 psum,
paired into [P, 2*QC] tiles for halved evacuation count), evacuations
alternating between ACT (activation-Copy) and DVE so neither becomes the
phase-1 bottleneck.  Input DMAs are chunked (wk et-split first, then x
token-slabs interleaved with wq/wv/wo) so the first K chain starts ~4us in.
The last 4 V chains are deferred into unit-0 segment slots of phase 2 to
keep the PE busy through the phase boundary (PE idle >3.4us triggers the
HAM half-clock throttle, measured 75-143us windows when it fires).

Phase 2 -- 32 units (head h, q-chunk of 512).  Per unit, 6 score segments
(3,3,3,3,2,2 k-tiles) write fp32 psum ([P, 3*QC] tiles, 2 buffers), then:
  - segments 0-3 are exp'd by ACT (exp-table preloaded, scale folded in),
  - segments 4-5 (4 k-tiles) by DVE via the Schraudolph bit-trick
    P16 = bitcast_fp16(int16(s*scale*1477.32 + 15315)), one fused
    tensor_scalar (DVE f32->int16 converts round-to-nearest; verified);
    splitting exp keeps the ~220us/core ACT exp floor off the critical
    path at a cost of ~8e-3 rel err.
  - AV (V-stationary, [65, 512] psum accumulated over 16 k-tiles) of the
    PREVIOUS unit is emitted before each score segment so the PE always
    has runnable work while a segment waits on exp.
  - normalization: recip of the denominator row (reciprocal_approx_fast
    on partition 0 -- the native reciprocal lowering costs ~3us/call),
    gpsimd partition_broadcast, DVE multiply into fp16 outT.
  - output-projection groups (yT[et, 512-token slice]) are interleaved
    one per unit as q-chunk blocks complete.

PSUM (start=True resets the WHOLE bank -- verified; single-region tiles
only): score segs 2x3 banks + AV accumulators 2x1 bank = 8 banks.

kernel() re-runs the device execution if the output is non-finite: the
first execution after NEFF load can race on cold SBUF (observed NaN on
first call, clean on retry).
"""

import math

import numpy as np

import concourse.bass as bass
import concourse.mybir as mybir
import concourse.tile as tile
from concourse import bacc
from concourse.bass import ds, ts
from concourse.bass_utils import run_bass_kernel_spmd

P = 128
T = 2048        # tokens per batch
DM = 1024       # model dim
E = 512         # per-core projection width (8 heads * 64)
H = 8           # heads per core
HD = 64
CD = DM // P    # fp16 contraction chunks over model dim
NKT = T // P    # k tiles per head
QC = 512        # q chunk
NQC = T // QC
F32 = mybir.dt.float32
FP16 = mybir.dt.float16
I16 = mybir.dt.int16
EXP = mybir.ActivationFunctionType.Exp
CPY = mybir.ActivationFunctionType.Copy

# exp split: trailing k-tiles per unit on the DVE Schraudolph path
DVE_KT = 4
SCHRAUD_A = 1024.0 / math.log(2.0)
SCHRAUD_B = 15360.0 - 45.0


def build(scale: float):
    nc = bacc.Bacc(None, target_bir_lowering=False, debug=False)
    xT = nc.declare_dram_parameter("xT", [DM, T], FP16, isOutput=False)
    wq = nc.declare_dram_parameter("wq", [DM, E], FP16, isOutput=False)
    wk = nc.declare_dram_parameter("wk", [DM, E], FP16, isOutput=False)
    wv = nc.declare_dram_parameter("wv", [DM, E], FP16, isOutput=False)
    wo = nc.declare_dram_parameter("wo", [P, 4, DM], FP16, isOutput=False)
    yT = nc.declare_dram_parameter("yT", [DM, T], F32, isOutput=True)

    with tile.TileContext(nc) as tc:
        with (
            tc.tile_pool(name="qk", bufs=1) as qkp,
            tc.tile_pool(name="vp", bufs=1) as vp,
            tc.tile_pool(name="misc", bufs=1) as miscp,
            tc.tile_pool(name="wts", bufs=1) as wp,
            tc.tile_pool(name="xt", bufs=1) as xp,
        ):
            QT = qkp.tile([P, 4, T], FP16, tag="QT")
            KT = qkp.tile([P, 4, T], FP16, tag="KT")
            V = vp.tile([P, NKT, H, HD + 1], FP16, tag="V")
            nc.vector.memset(V[:, :, :, HD : HD + 1], 1.0)
            # preload the exp table set so the first real exp doesn't stall
            warm = miscp.tile([1, 8], F32, tag="warm")
            nc.vector.memset(warm[:], 0.0)
            nc.scalar.activation(out=warm[:], in_=warm[:], func=EXP, scale=1.0)

            wq_sb = wp.tile([P, CD, E], FP16, tag="wq")
            wk_sb = wp.tile([P, CD, E], FP16, tag="wk")
            wv_sb = wp.tile([P, CD, E], FP16, tag="wv")
            wo_sb = wp.tile([P, 4, DM], FP16, tag="wo")
            xsb = xp.tile([P, CD, T], FP16, tag="x")

            def dma_x(tcl):
                for c in range(CD):
                    nc.sync.dma_start(
                        out=xsb[:, c, ts(tcl, QC)],
                        in_=xT[c * P : (c + 1) * P, ts(tcl, QC)])

            wk_r = wk.rearrange("(c p) e -> p c e", p=P)
            wq_r = wq.rearrange("(c p) e -> p c e", p=P)
            nc.sync.dma_start(out=wk_sb[:, :, ts(0, P)], in_=wk_r[:, :, ts(0, P)])
            dma_x(0)
            for et in range(1, 4):
                nc.sync.dma_start(out=wk_sb[:, :, ts(et, P)],
                                  in_=wk_r[:, :, ts(et, P)])
            nc.sync.dma_start(out=wq_sb[:], in_=wq_r)
            dma_x(1)
            nc.sync.dma_start(out=wv_sb[:], in_=wv.rearrange("(c p) e -> p c e", p=P))
            dma_x(2)
            nc.sync.dma_start(out=wo_sb[:], in_=wo[:])
            dma_x(3)

            # ---------------- phase 1: all projections -------------------
            with tc.tile_pool(name="p1", bufs=3, space="PSUM") as p1pool:
                ei = 0

                def evac(dstap, srcap):
                    nonlocal ei
                    ei += 1
                    if ei % 2:
                        nc.scalar.activation(out=dstap, in_=srcap, func=CPY)
                    else:
                        nc.vector.tensor_copy(dstap, srcap)

                for tc2 in range(2):
                    for wsb, dst in ((wk_sb, KT), (wq_sb, QT)):
                        for et in range(4):
                            ps = p1pool.tile([P, 2 * QC], F32, tag="pp")
                            for half in range(2):
                                for c in range(CD):
                                    nc.tensor.matmul(
                                        ps[:, half * QC : (half + 1) * QC],
                                        wsb[:, c, ts(et, P)],
                                        xsb[:, c, ts(2 * tc2 + half, QC)],
                                        start=(c == 0),
                                        stop=(c == CD - 1),
                                    )
                            evac(dst[:, et, ts(tc2, 2 * QC)], ps[:])
                for tt in range(NKT - 4):
                    ps = p1pool.tile([P, 2 * QC], F32, tag="pp")
                    for c in range(CD):
                        nc.tensor.matmul(
                            ps[:, 0:E],
                            xsb[:, c, ts(tt, P)],
                            wv_sb[:, c, :],
                            start=(c == 0),
                            stop=(c == CD - 1),
                        )
                    evac(V[:, tt, :, 0:HD],
                         ps[:, 0:E].rearrange("p (h e) -> p h e", h=H))

            # ---------------- phase 2 ------------------------------------
            with tc.tile_pool(name="outp", bufs=1) as outp:
                outT = outp.tile([P, 4, T], FP16, tag="outT")

                with (
                    tc.tile_pool(name="pbuf", bufs=3) as pbuf,
                    tc.tile_pool(name="nrm", bufs=2) as nrmp,
                    tc.tile_pool(name="yev", bufs=3) as yev,
                    tc.tile_pool(name="sps", bufs=2, space="PSUM") as spool,
                    tc.tile_pool(name="avp", bufs=2, space="PSUM") as avpool,
                ):
                    def emit_ygroup(et2, tcq):
                        # output projection yT[et2-tile, tcq-slice]
                        ps = spool.tile([P, 3 * QC], F32, tag="S")
                        for fc in range(4):
                            nc.tensor.matmul(
                                ps[:, 0:QC],
                                wo_sb[:, fc, ts(et2, P)],
                                outT[:, fc, ts(tcq, QC)],
                                start=(fc == 0),
                                stop=(fc == 3),
                            )
                        yt = yev.tile([P, QC], F32, tag="ye")
                        nc.vector.tensor_copy(yt[:], ps[:, 0:QC])
                        nc.sync.dma_start(
                            out=yT[et2 * P : (et2 + 1) * P, ts(tcq, QC)],
                            in_=yt[:],
                        )

                    SEGS = [(0, 3), (3, 6), (6, 9), (9, 12), (12, 14),
                            (14, 16)]
                    AVSPREAD = [3, 3, 3, 3, 2, 2]

                    def emit_score_seg(h, qc, k0, k1, Pu):
                        # k-tiles [k0,k1) of scores -> psum -> exp -> P
                        et, off = h // 2, (h % 2) * HD
                        n = k1 - k0
                        sp = spool.tile([P, 3 * QC], F32, tag="S")
                        for u in range(n):
                            kt = k0 + u
                            nc.tensor.matmul(
                                sp[:, u * QC : (u + 1) * QC],
                                KT[off : off + HD, et, kt * P : (kt + 1) * P],
                                QT[off : off + HD, et, ts(qc, QC)],
                                start=True,
                                stop=True,
                            )
                        if k0 >= NKT - DVE_KT:
                            # DVE Schraudolph exp: one fused tensor_scalar
                            nc.vector.tensor_scalar(
                                out=Pu[:, k0:k1, :].bitcast(I16),
                                in0=sp[:, 0 : n * QC].rearrange(
                                    "p (a b) -> p a b", b=QC),
                                scalar1=float(scale * SCHRAUD_A),
                                scalar2=float(SCHRAUD_B),
                                op0=mybir.AluOpType.mult,
                                op1=mybir.AluOpType.add,
                            )
                        else:
                            nc.scalar.activation(
                                out=Pu[:, k0:k1, :],
                                in_=sp[:, 0 : n * QC].rearrange(
                                    "p (a b) -> p a b", b=QC),
                                func=EXP,
                                scale=scale,
                            )

                    def emit_av(h, opsum, Pu, kt):
                        # V-stationary AV: out [65 feat, 512 q]; the 128-row
                        # weight load hides under the 512-cycle stream
                        nc.tensor.matmul(
                            opsum[0 : HD + 1, :],
                            V[:, kt, h, :],
                            Pu[:, kt, :],
                            start=(kt == 0),
                            stop=(kt == NKT - 1),
                        )

                    def emit_finish(opsum, h, qc):
                        # normalization: row HD of opsum is the denominator;
                        # recip on partition 0, gpsimd broadcast to HD
                        # partitions, multiply into feature-major outT
                        et, off = h // 2, (h % 2) * HD
                        den = nrmp.tile([1, QC], F32, tag="dn")
                        nc.vector.tensor_copy(den[:], opsum[HD : HD + 1, :])
                        recip = nrmp.tile([1, QC], F32, tag="rc")
                        nc.vector.reciprocal_approx_fast(recip[:], den[:])
                        bcs = nrmp.tile([HD, QC], F32, tag="bcs")
                        nc.gpsimd.partition_broadcast(bcs[:], recip[:],
                                                      channels=HD)
                        nc.vector.tensor_mul(
                            outT[off : off + HD, et, ts(qc, QC)],
                            opsum[0:HD, :],
                            bcs[:],
                        )

                    units = [(h, qc) for qc in range(NQC) for h in range(H)]
                    pending_y = []
                    deferred_v = list(range(NKT - 4, NKT))
                    prev = None  # (opsum, Pu, h, qc)
                    for ui, (h, qc) in enumerate(units):
                        Pu = pbuf.tile([P, NKT, QC], FP16, tag="P")
                        av_kt = 0
                        for si, (k0, k1) in enumerate(SEGS):
                            # AV of the previous unit first: it has no new
                            # dependencies, so the PE always has runnable
                            # work even while a score seg waits on exp
                            if prev is not None:
                                popsum, pPu, ph, pqc = prev
                                for _ in range(AVSPREAD[si]):
                                    emit_av(ph, popsum, pPu, av_kt)
                                    av_kt += 1
                            emit_score_seg(h, qc, k0, k1, Pu)
                            if deferred_v:
                                # fill the phase-boundary pipeline-fill gap
                                tt = deferred_v.pop(0)
                                ps = spool.tile([P, 3 * QC], F32, tag="S")
                                for c in range(CD):
                                    nc.tensor.matmul(
                                        ps[:, 0:E],
                                        xsb[:, c, ts(tt, P)],
                                        wv_sb[:, c, :],
                                        start=(c == 0),
                                        stop=(c == CD - 1),
                                    )
                                nc.vector.tensor_copy(
                                    V[:, tt, :, 0:HD],
                                    ps[:, 0:E].rearrange(
                                        "p (h e) -> p h e", h=H))
                            if si == 2 and pending_y:
                                emit_ygroup(*pending_y.pop(0))
                        if prev is not None:
                            popsum, pPu, ph, pqc = prev
                            emit_finish(popsum, ph, pqc)
                            if ph == H - 1:
                                pending_y.extend(
                                    (et2, pqc) for et2 in range(DM // P))
                        opsum = avpool.tile([P, QC], F32, tag="av")
                        prev = (opsum, Pu, h, qc)
                    popsum, pPu, ph, pqc = prev
                    for kt in range(NKT):
                        emit_av(ph, popsum, pPu, kt)
                    emit_finish(popsum, ph, pqc)
                    pending_y.extend((et2, pqc) for et2 in range(DM // P))
                    for et2, tcq in pending_y:
                        emit_ygroup(et2, tcq)

    nc.compile()
    return nc


_CACHE: dict = {}


def _get_program(scale: float):
    key = round(float(scale), 12)
    if key not in _CACHE:
        _CACHE[key] = build(key)
    return _CACHE[key]


def _make_in_maps(x, w_qkv, w_out):
    xTs = [np.ascontiguousarray(x[b].T).astype(np.float16) for b in range(4)]
    wslices = []
    for hg in range(2):
        sl = slice(hg * E, (hg + 1) * E)
        wo_h = np.ascontiguousarray(w_out[:, sl].T)  # [E, DM]
        wslices.append(
            {
                "wq": np.ascontiguousarray(
                    w_qkv[0 * DM :][sl, :].T).astype(np.float16),
                "wk": np.ascontiguousarray(
                    w_qkv[1 * DM :][sl, :].T).astype(np.float16),
                "wv": np.ascontiguousarray(
                    w_qkv[2 * DM :][sl, :].T).astype(np.float16),
                "wo": np.ascontiguousarray(
                    wo_h.reshape(4, P, DM).transpose(1, 0, 2)
                ).astype(np.float16),
            }
        )
    in_maps = []
    for c in range(8):
        b, hg = c // 2, c % 2
        m = {"xT": xTs[b]}
        m.update(wslices[hg])
        in_maps.append(m)
    return in_maps


def _execute(x, w_qkv, w_out, rescale, **spmd_kwargs):
    scale = float(np.asarray(rescale)) / math.sqrt(HD)
    nc = _get_program(scale)
    in_maps = _make_in_maps(x, w_qkv, w_out)
    return run_bass_kernel_spmd(nc, in_maps, list(range(8)), **spmd_kwargs)


def kernel(x, w_qkv, w_out, b_out, rescale):
    x = np.asarray(x, dtype=np.float32)
    w_qkv = np.asarray(w_qkv, dtype=np.float32)
    w_out = np.asarray(w_out, dtype=np.float32)
    b_out = np.asarray(b_out, dtype=np.float32)
    y = np.empty((4, T, DM), dtype=np.float32)
    # the very first execution after NEFF load can race on cold SBUF;
    # detect and re-run (subsequent executions are clean)
    for _attempt in range(3):
        res = _execute(x, w_qkv, w_out, rescale).results
        for b in range(4):
            acc = res[2 * b]["yT"] + res[2 * b + 1]["yT"]
            y[b] = acc.T + b_out
        if np.isfinite(y).all():
            break
    return y


# revision 5
# speedup vs baseline: 1.0055x; 1.0055x over previous
"""DriftAwareMultiHeadAttention on 8 Trainium2 NeuronCores.

Sharding (per spec hint): core c -> (batch b = c//2, head-group hg = c%2).
Each core: fp16 QKV projection (column-parallel over its 8 heads), full
attention for those heads, row-parallel partial output projection.
Host gather: y[b] = (yT[2b] + yT[2b+1]).T + b_out.

Measured on HW: ~400-405 us/core (vs 415-436 us baseline), rel err 9.4e-3.

Layout: feature-on-partition / token-on-free throughout (no on-chip
transposes).  Q^T/K^T [512, 2048] fp16 (head h -> e-tile h//2, partition
offset (h%2)*64); V [tokens, 8 heads x (64+1)] fp16 with a ones column so
the AV matmul emits the softmax denominator for free.

Phase 1 -- projections: fp16 chains (8 contraction chunks, N=512 psum,
paired into [P, 2*QC] tiles for halved evacuation count), evacuations
alternating between ACT (activation-Copy) and DVE so neither becomes the
phase-1 bottleneck.  Input DMAs are chunked (wk et-split first, then x
token-slabs interleaved with wq/wv/wo) so the first K chain starts ~4us in.
The last 4 V chains are deferred into unit-0 segment slots of phase 2 to
keep the PE busy through the phase boundary (PE idle >3.4us triggers the
HAM half-clock throttle, measured 75-143us windows when it fires).

Phase 2 -- 32 units (head h, q-chunk of 512).  Per unit, 6 score segments
(3,3,3,3,2,2 k-tiles) write fp32 psum ([P, 3*QC] tiles, 2 buffers), then:
  - segments 0-3 are exp'd by ACT (exp-table preloaded, scale folded in),
  - segments 4-5 (4 k-tiles) by DVE via the Schraudolph bit-trick
    P16 = bitcast_fp16(int16(s*scale*1477.32 + 15315)), one fused
    tensor_scalar (DVE f32->int16 converts round-to-nearest; verified);
    splitting exp keeps the ~220us/core ACT exp floor off the critical
    path at a cost of ~8e-3 rel err.
  - AV (V-stationary, [65, 512] psum accumulated over 16 k-tiles) of the
    PREVIOUS unit is emitted before each score segment so the PE always
    has runnable work while a segment waits on exp.
  - normalization: recip of the denominator row (reciprocal_approx_fast
    on partition 0 -- the native reciprocal lowering costs ~3us/call),
    gpsimd partition_broadcast, DVE multiply into fp16 outT.
  - output-projection groups (yT[et, 512-token slice]) are interleaved
    one per unit as q-chunk blocks complete.

PSUM (start=True resets the WHOLE bank -- verified; single-region tiles
only): score segs 2x3 banks + AV accumulators 2x1 bank = 8 banks.

kernel() re-runs the device execution if the output is non-finite: the
first execution after NEFF load can race on cold SBUF (observed NaN on
first call, clean on retry).
"""

import math

import numpy as np

import concourse.bass as bass
import concourse.mybir as mybir
import concourse.tile as tile
from concourse import bacc
from concourse.bass import ds, ts
from concourse.bass_utils import run_bass_kernel_spmd

P = 128
T = 2048        # tokens per batch
DM = 1024       # model dim
E = 512         # per-core projection width (8 heads * 64)
H = 8           # heads per core
HD = 64
CD = DM // P    # fp16 contraction chunks over model dim
NKT = T // P    # k tiles per head
QC = 512        # q chunk
NQC = T // QC
F32 = mybir.dt.float32
FP16 = mybir.dt.float16
I16 = mybir.dt.int16
EXP = mybir.ActivationFunctionType.Exp
CPY = mybir.ActivationFunctionType.Copy

# exp split: trailing k-tiles per unit on the DVE Schraudolph path
DVE_KT = 4
SCHRAUD_A = 1024.0 / math.log(2.0)
SCHRAUD_B = 15360.0 - 45.0


def build(scale: float):
    nc = bacc.Bacc(None, target_bir_lowering=False, debug=False)
    xT = nc.declare_dram_parameter("xT", [DM, T], FP16, isOutput=False)
    wq = nc.declare_dram_parameter("wq", [DM, E], FP16, isOutput=False)
    wk = nc.declare_dram_parameter("wk", [DM, E], FP16, isOutput=False)
    wv = nc.declare_dram_parameter("wv", [DM, E], FP16, isOutput=False)
    wo = nc.declare_dram_parameter("wo", [P, 4, DM], FP16, isOutput=False)
    yT = nc.declare_dram_parameter("yT", [DM, T], F32, isOutput=True)

    with tile.TileContext(nc) as tc:
        with (
            tc.tile_pool(name="qk", bufs=1) as qkp,
            tc.tile_pool(name="vp", bufs=1) as vp,
            tc.tile_pool(name="misc", bufs=1) as miscp,
            tc.tile_pool(name="wts", bufs=1) as wp,
            tc.tile_pool(name="xt", bufs=1) as xp,
        ):
            QT = qkp.tile([P, 4, T], FP16, tag="QT")
            KT = qkp.tile([P, 4, T], FP16, tag="KT")
            V = vp.tile([P, NKT, H, HD + 1], FP16, tag="V")
            nc.vector.memset(V[:, :, :, HD : HD + 1], 1.0)
            # preload the exp table set so the first real exp doesn't stall
            warm = miscp.tile([1, 8], F32, tag="warm")
            nc.vector.memset(warm[:], 0.0)
            nc.scalar.activation(out=warm[:], in_=warm[:], func=EXP, scale=1.0)

            wq_sb = wp.tile([P, CD, E], FP16, tag="wq")
            wk_sb = wp.tile([P, CD, E], FP16, tag="wk")
            wv_sb = wp.tile([P, CD, E], FP16, tag="wv")
            wo_sb = wp.tile([P, 4, DM], FP16, tag="wo")
            xsb = xp.tile([P, CD, T], FP16, tag="x")

            def dma_x(tcl):
                for c in range(CD):
                    nc.sync.dma_start(
                        out=xsb[:, c, ts(tcl, QC)],
                        in_=xT[c * P : (c + 1) * P, ts(tcl, QC)])

            wk_r = wk.rearrange("(c p) e -> p c e", p=P)
            wq_r = wq.rearrange("(c p) e -> p c e", p=P)
            nc.sync.dma_start(out=wk_sb[:, :, ts(0, P)], in_=wk_r[:, :, ts(0, P)])
            dma_x(0)
            for et in range(1, 4):
                nc.sync.dma_start(out=wk_sb[:, :, ts(et, P)],
                                  in_=wk_r[:, :, ts(et, P)])
            nc.sync.dma_start(out=wq_sb[:], in_=wq_r)
            dma_x(1)
            nc.sync.dma_start(out=wv_sb[:], in_=wv.rearrange("(c p) e -> p c e", p=P))
            dma_x(2)
            nc.sync.dma_start(out=wo_sb[:], in_=wo[:])
            dma_x(3)

            # ---------------- phase 1: all projections -------------------
            with tc.tile_pool(name="p1", bufs=3, space="PSUM") as p1pool:
                # spin the PE on dummy matmuls while the x DMA lands: keeps
                # every startup idle window under the ~3.4us HAM half-clock
                # trigger and finishes the p-state ramp before the first
                # real chain
                wa = miscp.tile([P, P], FP16, tag="wa")
                wb = miscp.tile([P, QC], FP16, tag="wb")
                nc.vector.memset(wa[:], 0.0)
                nc.vector.memset(wb[:], 0.0)
                wps = p1pool.tile([P, 2 * QC], F32, tag="pp")
                for _ in range(30):
                    nc.tensor.matmul(wps[:, 0:QC], wa[:], wb[:],
                                     start=True, stop=True)
                ei = 0

                def evac(dstap, srcap):
                    nonlocal ei
                    ei += 1
                    if ei % 2:
                        nc.scalar.activation(out=dstap, in_=srcap, func=CPY)
                    else:
                        nc.vector.tensor_copy(dstap, srcap)

                for tc2 in range(2):
                    for wsb, dst in ((wk_sb, KT), (wq_sb, QT)):
                        for et in range(4):
                            ps = p1pool.tile([P, 2 * QC], F32, tag="pp")
                            for half in range(2):
                                for c in range(CD):
                                    nc.tensor.matmul(
                                        ps[:, half * QC : (half + 1) * QC],
                                        wsb[:, c, ts(et, P)],
                                        xsb[:, c, ts(2 * tc2 + half, QC)],
                                        start=(c == 0),
                                        stop=(c == CD - 1),
                                    )
                            evac(dst[:, et, ts(tc2, 2 * QC)], ps[:])
                for tt in range(NKT - 4):
                    ps = p1pool.tile([P, 2 * QC], F32, tag="pp")
                    for c in range(CD):
                        nc.tensor.matmul(
                            ps[:, 0:E],
                            xsb[:, c, ts(tt, P)],
                            wv_sb[:, c, :],
                            start=(c == 0),
                            stop=(c == CD - 1),
                        )
                    evac(V[:, tt, :, 0:HD],
                         ps[:, 0:E].rearrange("p (h e) -> p h e", h=H))

            # ---------------- phase 2 ------------------------------------
            with tc.tile_pool(name="outp", bufs=1) as outp:
                outT = outp.tile([P, 4, T], FP16, tag="outT")

                with (
                    tc.tile_pool(name="pbuf", bufs=3) as pbuf,
                    tc.tile_pool(name="nrm", bufs=2) as nrmp,
                    tc.tile_pool(name="yev", bufs=3) as yev,
                    tc.tile_pool(name="sps", bufs=2, space="PSUM") as spool,
                    tc.tile_pool(name="avp", bufs=2, space="PSUM") as avpool,
                ):
                    def emit_ygroup(et2, tcq):
                        # output projection yT[et2-tile, tcq-slice]
                        ps = spool.tile([P, 3 * QC], F32, tag="S")
                        for fc in range(4):
                            nc.tensor.matmul(
                                ps[:, 0:QC],
                                wo_sb[:, fc, ts(et2, P)],
                                outT[:, fc, ts(tcq, QC)],
                                start=(fc == 0),
                                stop=(fc == 3),
                            )
                        yt = yev.tile([P, QC], F32, tag="ye")
                        nc.vector.tensor_copy(yt[:], ps[:, 0:QC])
                        nc.sync.dma_start(
                            out=yT[et2 * P : (et2 + 1) * P, ts(tcq, QC)],
                            in_=yt[:],
                        )

                    SEGS = [(0, 3), (3, 6), (6, 9), (9, 12), (12, 14),
                            (14, 16)]
                    AVSPREAD = [3, 3, 3, 3, 2, 2]

                    def emit_score_seg(h, qc, k0, k1, Pu):
                        # k-tiles [k0,k1) of scores -> psum -> exp -> P
                        et, off = h // 2, (h % 2) * HD
                        n = k1 - k0
                        sp = spool.tile([P, 3 * QC], F32, tag="S")
                        for u in range(n):
                            kt = k0 + u
                            nc.tensor.matmul(
                                sp[:, u * QC : (u + 1) * QC],
                                KT[off : off + HD, et, kt * P : (kt + 1) * P],
                                QT[off : off + HD, et, ts(qc, QC)],
                                start=True,
                                stop=True,
                            )
                        if k0 >= NKT - DVE_KT:
                            # DVE Schraudolph exp: one fused tensor_scalar
                            nc.vector.tensor_scalar(
                                out=Pu[:, k0:k1, :].bitcast(I16),
                                in0=sp[:, 0 : n * QC].rearrange(
                                    "p (a b) -> p a b", b=QC),
                                scalar1=float(scale * SCHRAUD_A),
                                scalar2=float(SCHRAUD_B),
                                op0=mybir.AluOpType.mult,
                                op1=mybir.AluOpType.add,
                            )
                        else:
                            nc.scalar.activation(
                                out=Pu[:, k0:k1, :],
                                in_=sp[:, 0 : n * QC].rearrange(
                                    "p (a b) -> p a b", b=QC),
                                func=EXP,
                                scale=scale,
                            )

                    def emit_av(h, opsum, Pu, kt):
                        # V-stationary AV: out [65 feat, 512 q]; the 128-row
                        # weight load hides under the 512-cycle stream
                        nc.tensor.matmul(
                            opsum[0 : HD + 1, :],
                            V[:, kt, h, :],
                            Pu[:, kt, :],
                            start=(kt == 0),
                            stop=(kt == NKT - 1),
                        )

                    def emit_finish(opsum, h, qc):
                        # normalization: row HD of opsum is the denominator;
                        # recip on partition 0, gpsimd broadcast to HD
                        # partitions, multiply into feature-major outT
                        et, off = h // 2, (h % 2) * HD
                        den = nrmp.tile([1, QC], F32, tag="dn")
                        nc.vector.tensor_copy(den[:], opsum[HD : HD + 1, :])
                        recip = nrmp.tile([1, QC], F32, tag="rc")
                        nc.vector.reciprocal_approx_fast(recip[:], den[:])
                        bcs = nrmp.tile([HD, QC], F32, tag="bcs")
                        nc.gpsimd.partition_broadcast(bcs[:], recip[:],
                                                      channels=HD)
                        nc.vector.tensor_mul(
                            outT[off : off + HD, et, ts(qc, QC)],
                            opsum[0:HD, :],
                            bcs[:],
                        )

                    units = [(h, qc) for qc in range(NQC) for h in range(H)]
                    pending_y = []
                    deferred_v = list(range(NKT - 4, NKT))
                    prev = None  # (opsum, Pu, h, qc)
                    for ui, (h, qc) in enumerate(units):
                        Pu = pbuf.tile([P, NKT, QC], FP16, tag="P")
                        av_kt = 0
                        for si, (k0, k1) in enumerate(SEGS):
                            # AV of the previous unit first: it has no new
                            # dependencies, so the PE always has runnable
                            # work even while a score seg waits on exp
                            if prev is not None:
                                popsum, pPu, ph, pqc = prev
                                for _ in range(AVSPREAD[si]):
                                    emit_av(ph, popsum, pPu, av_kt)
                                    av_kt += 1
                            emit_score_seg(h, qc, k0, k1, Pu)
                            if deferred_v:
                                # fill the phase-boundary pipeline-fill gap
                                tt = deferred_v.pop(0)
                                ps = spool.tile([P, 3 * QC], F32, tag="S")
                                for c in range(CD):
                                    nc.tensor.matmul(
                                        ps[:, 0:E],
                                        xsb[:, c, ts(tt, P)],
                                        wv_sb[:, c, :],
                                        start=(c == 0),
                                        stop=(c == CD - 1),
                                    )
                                nc.vector.tensor_copy(
                                    V[:, tt, :, 0:HD],
                                    ps[:, 0:E].rearrange(
                                        "p (h e) -> p h e", h=H))
                            if si == 2 and pending_y:
                                emit_ygroup(*pending_y.pop(0))
                        if prev is not None:
                            popsum, pPu, ph, pqc = prev
                            emit_finish(popsum, ph, pqc)
                            if ph == H - 1:
                                pending_y.extend(
                                    (et2, pqc) for et2 in range(DM // P))
                        opsum = avpool.tile([P, QC], F32, tag="av")
                        prev = (opsum, Pu, h, qc)
                    popsum, pPu, ph, pqc = prev
                    for kt in range(NKT):
                        emit_av(ph, popsum, pPu, kt)
                    emit_finish(popsum, ph, pqc)
                    pending_y.extend((et2, pqc) for et2 in range(DM // P))
                    for et2, tcq in pending_y:
                        emit_ygroup(et2, tcq)

    nc.compile()
    return nc


_CACHE: dict = {}


def _get_program(scale: float):
    key = round(float(scale), 12)
    if key not in _CACHE:
        _CACHE[key] = build(key)
    return _CACHE[key]


def _make_in_maps(x, w_qkv, w_out):
    xTs = [np.ascontiguousarray(x[b].T).astype(np.float16) for b in range(4)]
    wslices = []
    for hg in range(2):
        sl = slice(hg * E, (hg + 1) * E)
        wo_h = np.ascontiguousarray(w_out[:, sl].T)  # [E, DM]
        wslices.append(
            {
                "wq": np.ascontiguousarray(
                    w_qkv[0 * DM :][sl, :].T).astype(np.float16),
                "wk": np.ascontiguousarray(
                    w_qkv[1 * DM :][sl, :].T).astype(np.float16),
                "wv": np.ascontiguousarray(
                    w_qkv[2 * DM :][sl, :].T).astype(np.float16),
                "wo": np.ascontiguousarray(
                    wo_h.reshape(4, P, DM).transpose(1, 0, 2)
                ).astype(np.float16),
            }
        )
    in_maps = []
    for c in range(8):
        b, hg = c // 2, c % 2
        m = {"xT": xTs[b]}
        m.update(wslices[hg])
        in_maps.append(m)
    return in_maps


def _execute(x, w_qkv, w_out, rescale, **spmd_kwargs):
    scale = float(np.asarray(rescale)) / math.sqrt(HD)
    nc = _get_program(scale)
    in_maps = _make_in_maps(x, w_qkv, w_out)
    return run_bass_kernel_spmd(nc, in_maps, list(range(8)), **spmd_kwargs)


def kernel(x, w_qkv, w_out, b_out, rescale):
    x = np.asarray(x, dtype=np.float32)
    w_qkv = np.asarray(w_qkv, dtype=np.float32)
    w_out = np.asarray(w_out, dtype=np.float32)
    b_out = np.asarray(b_out, dtype=np.float32)
    y = np.empty((4, T, DM), dtype=np.float32)
    # the very first execution after NEFF load can race on cold SBUF;
    # detect and re-run (subsequent executions are clean)
    for _attempt in range(3):
        res = _execute(x, w_qkv, w_out, rescale).results
        for b in range(4):
            acc = res[2 * b]["yT"] + res[2 * b + 1]["yT"]
            y[b] = acc.T + b_out
        if np.isfinite(y).all():
            break
    return y


# revision 6
# speedup vs baseline: 1.0063x; 1.0007x over previous
"""DriftAwareMultiHeadAttention on 8 Trainium2 NeuronCores.

Sharding (per spec hint): core c -> (batch b = c//2, head-group hg = c%2).
Each core: fp16 QKV projection (column-parallel over its 8 heads), full
attention for those heads, row-parallel partial output projection.
Host gather: y[b] = (yT[2b] + yT[2b+1]).T + b_out.

Measured on HW: ~400-405 us/core (vs 415-436 us baseline), rel err 9.4e-3.

Layout: feature-on-partition / token-on-free throughout (no on-chip
transposes).  Q^T/K^T [512, 2048] fp16 (head h -> e-tile h//2, partition
offset (h%2)*64); V [tokens, 8 heads x (64+1)] fp16 with a ones column so
the AV matmul emits the softmax denominator for free.

Phase 1 -- projections: fp16 chains (8 contraction chunks, N=512 psum,
paired into [P, 2*QC] tiles for halved evacuation count), evacuations
alternating between ACT (activation-Copy) and DVE so neither becomes the
phase-1 bottleneck.  Input DMAs are chunked (wk et-split first, then x
token-slabs interleaved with wq/wv/wo) so the first K chain starts ~4us in.
The last 4 V chains are deferred into unit-0 segment slots of phase 2 to
keep the PE busy through the phase boundary (PE idle >3.4us triggers the
HAM half-clock throttle, measured 75-143us windows when it fires).

Phase 2 -- 32 units (head h, q-chunk of 512).  Per unit, 6 score segments
(3,3,3,3,2,2 k-tiles) write fp32 psum ([P, 3*QC] tiles, 2 buffers), then:
  - segments 0-3 are exp'd by ACT (exp-table preloaded, scale folded in),
  - segments 4-5 (4 k-tiles) by DVE via the Schraudolph bit-trick
    P16 = bitcast_fp16(int16(s*scale*1477.32 + 15315)), one fused
    tensor_scalar (DVE f32->int16 converts round-to-nearest; verified);
    splitting exp keeps the ~220us/core ACT exp floor off the critical
    path at a cost of ~8e-3 rel err.
  - AV (V-stationary, [65, 512] psum accumulated over 16 k-tiles) of the
    PREVIOUS unit is emitted before each score segment so the PE always
    has runnable work while a segment waits on exp.
  - normalization: recip of the denominator row (reciprocal_approx_fast
    on partition 0 -- the native reciprocal lowering costs ~3us/call),
    gpsimd partition_broadcast, DVE multiply into fp16 outT.
  - output-projection groups (yT[et, 512-token slice]) are interleaved
    one per unit as q-chunk blocks complete.

PSUM (start=True resets the WHOLE bank -- verified; single-region tiles
only): score segs 2x3 banks + AV accumulators 2x1 bank = 8 banks.

kernel() re-runs the device execution if the output is non-finite: the
first execution after NEFF load can race on cold SBUF (observed NaN on
first call, clean on retry).
"""

import math

import numpy as np

import concourse.bass as bass
import concourse.mybir as mybir
import concourse.tile as tile
from concourse import bacc
from concourse.bass import ds, ts
from concourse.bass_utils import run_bass_kernel_spmd

P = 128
T = 2048        # tokens per batch
DM = 1024       # model dim
E = 512         # per-core projection width (8 heads * 64)
H = 8           # heads per core
HD = 64
CD = DM // P    # fp16 contraction chunks over model dim
NKT = T // P    # k tiles per head
QC = 512        # q chunk
NQC = T // QC
F32 = mybir.dt.float32
FP16 = mybir.dt.float16
I16 = mybir.dt.int16
EXP = mybir.ActivationFunctionType.Exp
CPY = mybir.ActivationFunctionType.Copy

# exp split: trailing k-tiles per unit on the DVE Schraudolph path
DVE_KT = 4
SCHRAUD_A = 1024.0 / math.log(2.0)
SCHRAUD_B = 15360.0 - 45.0


def build(scale: float):
    nc = bacc.Bacc(None, target_bir_lowering=False, debug=False)
    xT = nc.declare_dram_parameter("xT", [DM, T], FP16, isOutput=False)
    wq = nc.declare_dram_parameter("wq", [DM, E], FP16, isOutput=False)
    wk = nc.declare_dram_parameter("wk", [DM, E], FP16, isOutput=False)
    wv = nc.declare_dram_parameter("wv", [DM, E], FP16, isOutput=False)
    wo = nc.declare_dram_parameter("wo", [P, 4, DM], FP16, isOutput=False)
    yT = nc.declare_dram_parameter("yT", [DM, T], F32, isOutput=True)

    with tile.TileContext(nc) as tc:
        with (
            tc.tile_pool(name="qk", bufs=1) as qkp,
            tc.tile_pool(name="vp", bufs=1) as vp,
            tc.tile_pool(name="misc", bufs=1) as miscp,
            tc.tile_pool(name="wts", bufs=1) as wp,
            tc.tile_pool(name="xt", bufs=1) as xp,
        ):
            QT = qkp.tile([P, 4, T], FP16, tag="QT")
            KT = qkp.tile([P, 4, T], FP16, tag="KT")
            V = vp.tile([P, NKT, H, HD + 1], FP16, tag="V")
            nc.vector.memset(V[:, :, :, HD : HD + 1], 1.0)
            # preload the exp table set so the first real exp doesn't stall
            warm = miscp.tile([1, 8], F32, tag="warm")
            nc.vector.memset(warm[:], 0.0)
            nc.scalar.activation(out=warm[:], in_=warm[:], func=EXP, scale=1.0)

            wq_sb = wp.tile([P, CD, E], FP16, tag="wq")
            wk_sb = wp.tile([P, CD, E], FP16, tag="wk")
            wv_sb = wp.tile([P, CD, E], FP16, tag="wv")
            wo_sb = wp.tile([P, 4, DM], FP16, tag="wo")
            xsb = xp.tile([P, CD, T], FP16, tag="x")

            def dma_x(tcl):
                for c in range(CD):
                    nc.sync.dma_start(
                        out=xsb[:, c, ts(tcl, QC)],
                        in_=xT[c * P : (c + 1) * P, ts(tcl, QC)])

            wk_r = wk.rearrange("(c p) e -> p c e", p=P)
            wq_r = wq.rearrange("(c p) e -> p c e", p=P)
            # the first K chain (paired 2*QC) consumes x slabs t0 AND t1,
            # so both must land before wq/wv/wo
            nc.sync.dma_start(out=wk_sb[:, :, ts(0, P)], in_=wk_r[:, :, ts(0, P)])
            dma_x(0)
            dma_x(1)
            for et in range(1, 4):
                nc.sync.dma_start(out=wk_sb[:, :, ts(et, P)],
                                  in_=wk_r[:, :, ts(et, P)])
            nc.sync.dma_start(out=wq_sb[:], in_=wq_r)
            nc.sync.dma_start(out=wv_sb[:], in_=wv.rearrange("(c p) e -> p c e", p=P))
            dma_x(2)
            dma_x(3)
            nc.sync.dma_start(out=wo_sb[:], in_=wo[:])

            # ---------------- phase 1: all projections -------------------
            with tc.tile_pool(name="p1", bufs=3, space="PSUM") as p1pool:
                # spin the PE on dummy matmuls while the x DMA lands: keeps
                # every startup idle window under the ~3.4us HAM half-clock
                # trigger and finishes the p-state ramp before the first
                # real chain
                wa = miscp.tile([P, P], FP16, tag="wa")
                wb = miscp.tile([P, QC], FP16, tag="wb")
                nc.vector.memset(wa[:], 0.0)
                nc.vector.memset(wb[:], 0.0)
                wps = p1pool.tile([P, 2 * QC], F32, tag="pp")
                for _ in range(30):
                    nc.tensor.matmul(wps[:, 0:QC], wa[:], wb[:],
                                     start=True, stop=True)
                ei = 0

                def evac(dstap, srcap):
                    nonlocal ei
                    ei += 1
                    if ei % 2:
                        nc.scalar.activation(out=dstap, in_=srcap, func=CPY)
                    else:
                        nc.vector.tensor_copy(dstap, srcap)

                for tc2 in range(2):
                    for wsb, dst in ((wk_sb, KT), (wq_sb, QT)):
                        for et in range(4):
                            ps = p1pool.tile([P, 2 * QC], F32, tag="pp")
                            for half in range(2):
                                for c in range(CD):
                                    nc.tensor.matmul(
                                        ps[:, half * QC : (half + 1) * QC],
                                        wsb[:, c, ts(et, P)],
                                        xsb[:, c, ts(2 * tc2 + half, QC)],
                                        start=(c == 0),
                                        stop=(c == CD - 1),
                                    )
                            evac(dst[:, et, ts(tc2, 2 * QC)], ps[:])
                for tt in range(NKT - 4):
                    ps = p1pool.tile([P, 2 * QC], F32, tag="pp")
                    for c in range(CD):
                        nc.tensor.matmul(
                            ps[:, 0:E],
                            xsb[:, c, ts(tt, P)],
                            wv_sb[:, c, :],
                            start=(c == 0),
                            stop=(c == CD - 1),
                        )
                    evac(V[:, tt, :, 0:HD],
                         ps[:, 0:E].rearrange("p (h e) -> p h e", h=H))

            # ---------------- phase 2 ------------------------------------
            with tc.tile_pool(name="outp", bufs=1) as outp:
                outT = outp.tile([P, 4, T], FP16, tag="outT")

                with (
                    tc.tile_pool(name="pbuf", bufs=3) as pbuf,
                    tc.tile_pool(name="nrm", bufs=2) as nrmp,
                    tc.tile_pool(name="yev", bufs=3) as yev,
                    tc.tile_pool(name="sps", bufs=2, space="PSUM") as spool,
                    tc.tile_pool(name="avp", bufs=2, space="PSUM") as avpool,
                ):
                    def emit_ygroup(et2, tcq):
                        # output projection yT[et2-tile, tcq-slice]
                        ps = spool.tile([P, 3 * QC], F32, tag="S")
                        for fc in range(4):
                            nc.tensor.matmul(
                                ps[:, 0:QC],
                                wo_sb[:, fc, ts(et2, P)],
                                outT[:, fc, ts(tcq, QC)],
                                start=(fc == 0),
                                stop=(fc == 3),
                            )
                        yt = yev.tile([P, QC], F32, tag="ye")
                        nc.vector.tensor_copy(yt[:], ps[:, 0:QC])
                        nc.sync.dma_start(
                            out=yT[et2 * P : (et2 + 1) * P, ts(tcq, QC)],
                            in_=yt[:],
                        )

                    SEGS = [(0, 3), (3, 6), (6, 9), (9, 12), (12, 14),
                            (14, 16)]
                    AVSPREAD = [3, 3, 3, 3, 2, 2]

                    def emit_score_seg(h, qc, k0, k1, Pu):
                        # k-tiles [k0,k1) of scores -> psum -> exp -> P
                        et, off = h // 2, (h % 2) * HD
                        n = k1 - k0
                        sp = spool.tile([P, 3 * QC], F32, tag="S")
                        for u in range(n):
                            kt = k0 + u
                            nc.tensor.matmul(
                                sp[:, u * QC : (u + 1) * QC],
                                KT[off : off + HD, et, kt * P : (kt + 1) * P],
                                QT[off : off + HD, et, ts(qc, QC)],
                                start=True,
                                stop=True,
                            )
                        if k0 >= NKT - DVE_KT:
                            # DVE Schraudolph exp: one fused tensor_scalar
                            nc.vector.tensor_scalar(
                                out=Pu[:, k0:k1, :].bitcast(I16),
                                in0=sp[:, 0 : n * QC].rearrange(
                                    "p (a b) -> p a b", b=QC),
                                scalar1=float(scale * SCHRAUD_A),
                                scalar2=float(SCHRAUD_B),
                                op0=mybir.AluOpType.mult,
                                op1=mybir.AluOpType.add,
                            )
                        else:
                            nc.scalar.activation(
                                out=Pu[:, k0:k1, :],
                                in_=sp[:, 0 : n * QC].rearrange(
                                    "p (a b) -> p a b", b=QC),
                                func=EXP,
                                scale=scale,
                            )

                    def emit_av(h, opsum, Pu, kt):
                        # V-stationary AV: out [65 feat, 512 q]; the 128-row
                        # weight load hides under the 512-cycle stream
                        nc.tensor.matmul(
                            opsum[0 : HD + 1, :],
                            V[:, kt, h, :],
                            Pu[:, kt, :],
                            start=(kt == 0),
                            stop=(kt == NKT - 1),
                        )

                    def emit_finish(opsum, h, qc):
                        # normalization: row HD of opsum is the denominator;
                        # recip on partition 0, gpsimd broadcast to HD
                        # partitions, multiply into feature-major outT
                        et, off = h // 2, (h % 2) * HD
                        den = nrmp.tile([1, QC], F32, tag="dn")
                        nc.vector.tensor_copy(den[:], opsum[HD : HD + 1, :])
                        recip = nrmp.tile([1, QC], F32, tag="rc")
                        nc.vector.reciprocal_approx_fast(recip[:], den[:])
                        bcs = nrmp.tile([HD, QC], F32, tag="bcs")
                        nc.gpsimd.partition_broadcast(bcs[:], recip[:],
                                                      channels=HD)
                        nc.vector.tensor_mul(
                            outT[off : off + HD, et, ts(qc, QC)],
                            opsum[0:HD, :],
                            bcs[:],
                        )

                    units = [(h, qc) for qc in range(NQC) for h in range(H)]
                    pending_y = []
                    deferred_v = list(range(NKT - 4, NKT))
                    prev = None  # (opsum, Pu, h, qc)
                    for ui, (h, qc) in enumerate(units):
                        Pu = pbuf.tile([P, NKT, QC], FP16, tag="P")
                        av_kt = 0
                        for si, (k0, k1) in enumerate(SEGS):
                            # AV of the previous unit first: it has no new
                            # dependencies, so the PE always has runnable
                            # work even while a score seg waits on exp
                            if prev is not None:
                                popsum, pPu, ph, pqc = prev
                                for _ in range(AVSPREAD[si]):
                                    emit_av(ph, popsum, pPu, av_kt)
                                    av_kt += 1
                            emit_score_seg(h, qc, k0, k1, Pu)
                            if deferred_v:
                                # fill the phase-boundary pipeline-fill gap
                                tt = deferred_v.pop(0)
                                ps = spool.tile([P, 3 * QC], F32, tag="S")
                                for c in range(CD):
                                    nc.tensor.matmul(
                                        ps[:, 0:E],
                                        xsb[:, c, ts(tt, P)],
                                        wv_sb[:, c, :],
                                        start=(c == 0),
                                        stop=(c == CD - 1),
                                    )
                                nc.vector.tensor_copy(
                                    V[:, tt, :, 0:HD],
                                    ps[:, 0:E].rearrange(
                                        "p (h e) -> p h e", h=H))
                            if si == 2 and pending_y:
                                emit_ygroup(*pending_y.pop(0))
                        if prev is not None:
                            popsum, pPu, ph, pqc = prev
                            emit_finish(popsum, ph, pqc)
                            if ph == H - 1:
                                pending_y.extend(
                                    (et2, pqc) for et2 in range(DM // P))
                        opsum = avpool.tile([P, QC], F32, tag="av")
                        prev = (opsum, Pu, h, qc)
                    popsum, pPu, ph, pqc = prev
                    for kt in range(NKT):
                        emit_av(ph, popsum, pPu, kt)
                    emit_finish(popsum, ph, pqc)
                    pending_y.extend((et2, pqc) for et2 in range(DM // P))
                    for et2, tcq in pending_y:
                        emit_ygroup(et2, tcq)

    nc.compile()
    return nc


_CACHE: dict = {}


def _get_program(scale: float):
    key = round(float(scale), 12)
    if key not in _CACHE:
        _CACHE[key] = build(key)
    return _CACHE[key]


def _make_in_maps(x, w_qkv, w_out):
    xTs = [np.ascontiguousarray(x[b].T).astype(np.float16) for b in range(4)]
    wslices = []
    for hg in range(2):
        sl = slice(hg * E, (hg + 1) * E)
        wo_h = np.ascontiguousarray(w_out[:, sl].T)  # [E, DM]
        wslices.append(
            {
                "wq": np.ascontiguousarray(
                    w_qkv[0 * DM :][sl, :].T).astype(np.float16),
                "wk": np.ascontiguousarray(
                    w_qkv[1 * DM :][sl, :].T).astype(np.float16),
                "wv": np.ascontiguousarray(
                    w_qkv[2 * DM :][sl, :].T).astype(np.float16),
                "wo": np.ascontiguousarray(
                    wo_h.reshape(4, P, DM).transpose(1, 0, 2)
                ).astype(np.float16),
            }
        )
    in_maps = []
    for c in range(8):
        b, hg = c // 2, c % 2
        m = {"xT": xTs[b]}
        m.update(wslices[hg])
        in_maps.append(m)
    return in_maps


def _execute(x, w_qkv, w_out, rescale, **spmd_kwargs):
    scale = float(np.asarray(rescale)) / math.sqrt(HD)
    nc = _get_program(scale)
    in_maps = _make_in_maps(x, w_qkv, w_out)
    return run_bass_kernel_spmd(nc, in_maps, list(range(8)), **spmd_kwargs)


def kernel(x, w_qkv, w_out, b_out, rescale):
    x = np.asarray(x, dtype=np.float32)
    w_qkv = np.asarray(w_qkv, dtype=np.float32)
    w_out = np.asarray(w_out, dtype=np.float32)
    b_out = np.asarray(b_out, dtype=np.float32)
    y = np.empty((4, T, DM), dtype=np.float32)
    # the very first execution after NEFF load can race on cold SBUF;
    # detect and re-run (subsequent executions are clean)
    for _attempt in range(3):
        res = _execute(x, w_qkv, w_out, rescale).results
        for b in range(4):
            acc = res[2 * b]["yT"] + res[2 * b + 1]["yT"]
            y[b] = acc.T + b_out
        if np.isfinite(y).all():
            break
    return y
